# revision 31
# baseline (speedup 1.0000x reference)
"""Trainium2 kernel: 4096x4096 single-channel 7x7 valid cross-correlation + bias.

Final strategy (v17): 4x concurrent 64x64 PE tiles + write-stream pipeline
--------------------------------------------------------------------------
Decomposition: y[r,c] = sum_j sum_i W[i,j] x[r+i, c+j] as 7 banded-Toeplitz
matmuls (one per kernel column j) accumulated in PSUM.  Strips of 64 input
rows -> 58 output rows; contraction K=64 = strip rows; lhsT [64, 58] = T_j
(band W[u-m, j]); the j-shift is a free-dim slice of the moving operand.
4 concurrent 64x64 PE sub-arrays (tile_position) quadruple the banded
matmul density; 18 rounds x 28 matmuls, N=512.

HW model measured on this part (probe + traces):
- Tiled matmuls are paced by the serialized LDWEIGHTS stream (~cols/1.2GHz
  + ~12ns per MM, LDW:MM is 1:1, no reuse/elision).  M=58 maximizes output
  rows per LDW-time; rounds run at 1.51us steady (28 MMs).
- SBUF->DRAM writes share one ~105 GB/s resource per core (parallel store
  queues DEGRADE to ~70 total), so the 4.26 MB output is written from one
  queue (gpsimd) as a continuous stream of 8KB-max packets; pair-packed
  [128, 4*512] staging keeps lines at the packet-size cap.  Total time ~=
  first-store + output_bytes/105GB/s: the write stream is the binding
  constraint, so stores start as early as drains allow.
- DMA access patterns: only the outermost dim maps to partitions; DRAM-side
  dims must stay flat/contiguous per line or descriptors fragment (1KB
  packets).  Wide 128-partition transfers are ~3x faster than 32-wide.
- All input tiles + staging tiles stay resident in SBUF (no pool recycling
  -> no false cross-round dependencies).  7 full-array warmup matmuls
  during the load head take the PE HAM clock-gate to 2.4 GHz before the
  real rounds.  Drains alternate ScalarE/VectorE; loads own the sync queue.

Sharding: output columns across 8 cores (512 each + 6-col halo host-side).
Per-core HW time ~57us (baseline 83us).
"""

import os

import numpy as np
import ml_dtypes

import concourse.bass as bass
import concourse.bacc as bacc_mod
import concourse.mybir as mybir
import concourse.tile as tile
from concourse.bass_utils import run_bass_kernel_spmd

H = 4096          # input rows
W = 4096          # input cols
KH = 7            # kernel rows
KW = 7            # kernel cols
OH = H - KH + 1   # 4090 output rows
OW = W - KW + 1   # 4090 output cols
NCORES = 8
CW = 512          # output cols per core
SW = CW + KW - 1  # 518 input cols per shard

TS = 64           # input rows per strip
SOUT = TS - KH + 1  # 58 output rows per strip
NROUNDS = 18      # 4 strips per round
NSTRIPS = 4 * NROUNDS           # 72 strip slots (71 real)
FREEW = 1040                    # 2 slots x 518 + pad (2080B lines)
PAD_ROWS = SOUT * (NSTRIPS - 1) + TS  # 4182

_BF16 = ml_dtypes.bfloat16


def _build_program(bias_val: float) -> bass.Bass:
    nc = bacc_mod.Bacc("TRN2", target_bir_lowering=False)

    x_d = nc.dram_tensor("xs", [NROUNDS, 128, FREEW], mybir.dt.bfloat16,
                         kind="ExternalInput")
    w_d = nc.dram_tensor("tmat", [128, KW * SOUT], mybir.dt.bfloat16,
                         kind="ExternalInput")
    # Output pair blocks (pair P = rounds 2P..2P+1, pos = 2*(R%2)+r2,
    # strip s = 4R + 2r2 + c2, output row 58s + q), contiguous per store.
    # [<=58-partition, 4KB-line] stores are the only shape that does NOT
    # slow concurrent matmuls (~145 GB/s; wider partitions or longer
    # lines cost the PE ~20%).  The first two pairs are stored as four
    # 29-line DMAs each: the tile framework recycles ~8 DMA-completion
    # semaphores and the queue completes breadth-first, so with 2-DMA
    # pairs the 9th DMA (pair 4) stalls ~5us on pair 0's completion and
    # the ring runs dry; quartering the early pairs shifts the semaphore
    # window so reuse always lands on long-completed DMAs.
    y_d = nc.dram_tensor("y", [2, NROUNDS // 2, SOUT, 4 * CW],
                         mybir.dt.bfloat16, kind="ExternalOutput")

    with tile.TileContext(nc) as tc:
        with (
            tc.tile_pool(name="const", bufs=1) as constp,
            tc.tile_pool(name="xg", bufs=NROUNDS) as xgp,
            tc.tile_pool(name="yg", bufs=NROUNDS) as ygp,
            tc.tile_pool(name="ps", bufs=8, space="PSUM") as psp,
        ):
            w_sb = constp.tile([128, KW * SOUT], mybir.dt.bfloat16)
            nc.sync.dma_start(w_sb[:, :], w_d[:, :])

            # No warmup matmuls: the HAM clock reaches full duty at a fixed
            # ~13.4us after exec start regardless of PE activity (measured
            # across runs), so warmups only delay round 0.

            xg_tiles = {}

            def load_round(R, eng):
                xg = xgp.tile([128, FREEW], mybir.dt.bfloat16,
                              name="xg", tag="xg")
                xg_tiles[R] = xg
                eng.dma_start(xg[:, :], x_d[R])

            # x0/x1 ride the sync queue alone (w + x0 + x1 only) so x0's
            # completion isn't deferred behind later loads (the queue
            # completes breadth-first); x2+ go to the scalar HW-DGE queue,
            # which is fast in the DRAM->SBUF direction.
            load_round(0, nc.sync)
            load_round(1, nc.sync)

            yt_pair = None
            for R in range(NROUNDS):
                if R + 2 < NROUNDS:
                    load_round(R + 2, nc.scalar)
                xg = xg_tiles.pop(R)
                ps_tiles = [psp.tile([128, CW], mybir.dt.float32,
                                     name="ps", tag="ps")
                            for _ in range(2)]

                for j in range(KW):
                    for r2 in range(2):
                        for c2 in range(2):
                            if R == NROUNDS - 1 and r2 == 1 and c2 == 1:
                                continue  # strip 71 is past row 4089
                            off = c2 * SW + j
                            nc.tensor.matmul(
                                ps_tiles[r2][64 * c2:64 * c2 + SOUT, :],
                                w_sb[64 * r2:64 * r2 + 64,
                                     SOUT * j:SOUT * j + SOUT],
                                xg[64 * r2:64 * r2 + 64, off:off + CW],
                                start=(j == 0),
                                stop=(j == KW - 1),
                                tile_position=(64 * r2, 64 * c2),
                            )

                rq = R % 2
                if rq == 0:
                    yt = ygp.tile([128, 4 * CW], mybir.dt.bfloat16,
                                  name="yg", tag="yg")
                for r2 in range(2):
                    dst = yt[:, (2 * rq + r2) * CW:(2 * rq + r2 + 1) * CW]
                    src = ps_tiles[r2][:, :]
                    if R % 2 == 0:
                        nc.scalar.activation(
                            dst, src, mybir.ActivationFunctionType.Copy,
                            bias=float(bias_val),
                        )
                    else:
                        nc.vector.tensor_scalar_add(dst, src, float(bias_val))

                # pair stores on the gpsimd SWDGE queue — the only fast
                # store path (HW-DGE queues cap ~50 GB/s for SBUF->DRAM).
                if rq == 1:
                    P = R // 2
                    half = 29
                    for c2 in range(2):
                        if P < 2:  # quartered: see y_d comment
                            for h in range(2):
                                nc.gpsimd.dma_start(
                                    y_d[c2, P, h * half:(h + 1) * half],
                                    yt[64 * c2 + h * half:
                                       64 * c2 + (h + 1) * half, :])
                        else:
                            nc.gpsimd.dma_start(
                                y_d[c2, P],
                                yt[64 * c2:64 * c2 + SOUT, :])

    nc.compile()
    nc.finalize()
    return nc


def _toeplitz(weight: np.ndarray) -> np.ndarray:
    """[128, 7*58] bf16: block j holds T_j[u, m] = W[u-m, j] (band 0<=u-m<7),
    u in [0,64), m in [0,58); replicated for partition half r2=1."""
    t = np.zeros((TS, KW * SOUT), np.float32)
    for j in range(KW):
        for i in range(KH):
            mm = np.arange(0, SOUT)
            t[mm + i, j * SOUT + mm] = weight[i, j]
    return np.tile(t, (2, 1)).astype(_BF16)


def _pack_shard(x_bf: np.ndarray, c0: int) -> np.ndarray:
    """[18, 128, 1040] bf16: partition 64*r2 + p of round R, free slot c2
    holds row 58*(4R + 2*r2 + c2) + p."""
    valid = min(SW, W - c0)
    xs = np.zeros((PAD_ROWS, SW), _BF16)
    xs[:H, :valid] = x_bf[:, c0:c0 + valid]
    R = np.arange(NROUNDS)
    out = np.zeros((NROUNDS, 128, FREEW), _BF16)
    for r2 in range(2):
        for c2 in range(2):
            s = 4 * R + 2 * r2 + c2
            rows = SOUT * s[:, None] + np.arange(TS)[None, :]
            out[:, 64 * r2:64 * r2 + TS, c2 * SW:(c2 + 1) * SW] = xs[rows]
    return out


def _unpack_out(y_packed: np.ndarray) -> np.ndarray:
    """[2, 9, 58, 4*512] bf16 -> [4090, 512] f32 (strip 8P+4rq+2r2+c2)."""
    y = y_packed.reshape(2, NROUNDS // 2, SOUT, 2, 2, CW)  # [c2,P,q,rq,r2,w]
    y = y.transpose(1, 3, 4, 0, 2, 5)                      # [P,rq,r2,c2,q,w]
    return y.reshape(NSTRIPS * SOUT, CW)[:OH].astype(np.float32)


def kernel(x: np.ndarray, weight: np.ndarray, bias: np.ndarray) -> np.ndarray:
    x = np.asarray(x, dtype=np.float32)
    weight = np.asarray(weight, dtype=np.float32)
    bias = np.asarray(bias, dtype=np.float32)

    tmat = _toeplitz(weight)
    x_bf = x.astype(_BF16)

    in_maps = []
    for c in range(NCORES):
        in_maps.append({"xs": _pack_shard(x_bf, CW * c), "tmat": tmat})

    nc = _build_program(float(bias[0]))

    trace = bool(int(os.environ.get("CONV_KERNEL_TRACE", "0")))
    res = run_bass_kernel_spmd(nc, in_maps, core_ids=list(range(NCORES)),
                               trace=trace)
    if trace:
        kernel.last_exec_time_ns = res.exec_time_ns

    cols = []
    for c in range(NCORES):
        valid_out = min(CW, OW - CW * c)
        cols.append(_unpack_out(np.asarray(res.results[c]["y"]))[:, :valid_out])
    return np.concatenate(cols, axis=1).astype(np.float32)

